# revision 1
# baseline (speedup 1.0000x reference)
"""MemoryNet kernel for 8 TRN2 NeuronCores (Bass/Tile).

Reference (single-device):
    key = softmax(mem @ fk_w.T + fk_b, axis=-1)      # [J, D]
    val = relu(mem @ fv_w.T + fv_b)                  # [J, D]
    att = softmax(k @ key.T, axis=-1)                # [N, J]
    out = att @ val                                  # [N, D]
with J=4096 (num_mem), MD=512 (mem_dim), D=1024 (inp_dim), N=32768.

Sharding: data-parallel over rows of k (N) across 8 cores; mem + weights
replicated on every core. Each core computes out rows for its shard; the
host concatenates.

Per-core algorithm. Derivation matmuls in bf16; the two big attention
matmuls run in fp8e4m3 with perf_mode=DoubleRow (2 contraction rows per
PE cell). fp8's ~6% relative steps would destroy att's small softmax
deviations if E=exp(s)~1.0 were quantized directly, so we store
Es = E - 1 (|Es|~0.04, 12x better absolute precision) and reconstruct:
    out = (colsum(val) + Es @ val) / (J + Es @ 1)
which matches full-bf16 accuracy (~6e-4 scale-relative, measured).

  Phase 0 (replicated derivation):
    memT/fk_wT/fv_wT via bf16 PE transposes.
    ekT[d,j]  = exp(fk_wT.T @ memT + fk_b[d])  -> fp8, d-pair-interleaved
    c[j]      = 1 / sum_d ekT   (ones matmul + transposed reciprocal)
    val[j,dd] = relu(memT.T @ fv_wT + fv_b)    -> fp8 (+ bf16 temp for
                valsum), fv_b added via rank-1 matmul
    valsum    = colsum(val)  (fp32 psum), broadcast to [128, D]
  Phase 1 (per chunk of NCHUNK k-rows):
    kT8[d,n]  via fp8 PE transposes of DMA'd k rows
    u[j-tile, n] = ekT8.T @ kT8      (DoubleRow, PSUM accumulate over d)
    exp in-place on PSUM (ACT, per-partition scale c_j), then
    Es = u - 1 -> fp8 SBUF (DVE)
    num[n-tile, dd] = Es.T @ val     (DoubleRow, accumulate over j)
    den[n-tile, 1]  = Es.T @ ones    (same lhsT, free-dim-1 matmul)
    out = (num + valsum) * 1/(J + den)
"""

import numpy as np

P = 128
J = 4096      # num_mem
MD = 512      # mem_dim
D = 1024      # inp_dim
NTOT = 32768  # total k rows
NCORES = 8
S = NTOT // NCORES   # k rows per core
NCHUNK = 512         # k rows processed per phase-1 chunk

_CACHE = {}


def _build():
    import concourse.bass as bass
    import concourse.tile as tile
    from concourse import bacc, mybir

    f32 = mybir.dt.float32
    bf16 = mybir.dt.bfloat16
    fp8 = mybir.dt.float8e4
    DR = mybir.MatmulPerfMode.DoubleRow
    AF = mybir.ActivationFunctionType

    nc = bacc.Bacc("TRN2", target_bir_lowering=False, debug=False,
                   num_devices=NCORES)

    kt_d = nc.dram_tensor("kt8", [D, S], fp8, kind="ExternalInput").ap()
    memt16_d = nc.dram_tensor("memt16", [MD, J], bf16, kind="ExternalInput").ap()
    memt8_d = nc.dram_tensor("memt8", [MD, J], fp8, kind="ExternalInput").ap()
    fkwt_d = nc.dram_tensor("fkwt8", [MD, D], fp8, kind="ExternalInput").ap()
    fkb_d = nc.dram_tensor("fk_b", [D], f32, kind="ExternalInput").ap()
    fvwt_d = nc.dram_tensor("fvwt16", [MD, D], bf16, kind="ExternalInput").ap()
    fvb_d = nc.dram_tensor("fv_b", [D], f32, kind="ExternalInput").ap()
    id_d = nc.dram_tensor("ident", [P, P], f32, kind="ExternalInput").ap()
    out_d = nc.dram_tensor("out", [S, D], f32, kind="ExternalOutput").ap()

    JT = J // P        # 32 j-tiles
    DT = D // P        # 8 d-tiles
    MT = MD // P       # 4 m-tiles
    NC_ = S // NCHUNK  # phase-1 chunks
    NS = NCHUNK // P   # n-subtiles per chunk

    with tile.TileContext(nc) as tc:
        from contextlib import ExitStack
        ctx = ExitStack()
        with ctx:
            persist = ctx.enter_context(tc.tile_pool(name="persist", bufs=1))
            ps_s = ctx.enter_context(tc.tile_pool(name="ps_s", bufs=4, space="PSUM"))
            ps_d = ctx.enter_context(tc.tile_pool(name="ps_d", bufs=1, space="PSUM"))

            # persistent tiles.  fp8 operands for DoubleRow matmuls are laid
            # out pair-interleaved: plane [.., i2, o, ..] holds contraction
            # row 256*i2 + 128*o + p.
            ekT8 = persist.tile([P, DT // 2, 2, J], fp8, tag="ekT8")
            val8 = persist.tile([P, JT // 2, 2, D], fp8, tag="val8")
            vsum_bc = persist.tile([P, D], f32, tag="vsum_bc")
            ident = persist.tile([P, P], f32, tag="ident")
            ones_c16 = persist.tile([P, 1], bf16, tag="ones_c")   # colsum lhsT
            ones_c8 = persist.tile([P, 1], fp8, tag="ones_c8")    # fp8 colsum lhsT
            ones8 = persist.tile([P, 2, 16], fp8, tag="ones8")    # DR den rhs
            ones_r16 = persist.tile([1, P], bf16, tag="ones_r")   # rank-1 bias lhsT
            ones_r32 = persist.tile([1, P], f32, tag="ones_r32")  # rank-1 f32 lhsT
            fkbT = persist.tile([P, DT], f32, tag="fkbT")
            c_col = persist.tile([P, JT], f32, tag="c_col")       # 1/keysum per j

            nc.sync.dma_start(out=ident, in_=id_d)
            nc.vector.memset(ones_c16, 1.0)
            nc.vector.memset(ones_c8, 1.0)
            nc.vector.memset(ones8, 1.0)
            nc.vector.memset(ones_r16, 1.0)
            nc.vector.memset(ones_r32, 1.0)
            # fk_b -> per-partition layout: fkbT[p, t] = fk_b[t*128 + p]
            # (gpsimd queue: 1024 4-byte descriptors would stall the sync
            # queue ahead of the weight loads)
            nc.gpsimd.dma_start(out=fkbT,
                                in_=fkb_d.rearrange("(t p) -> p t", p=P))

            # k-chunk load/cast/transpose chain.  Defined up front so
            # chunk 0 can be prefetched during phase 0 (it has no dependency
            # on the derivation).
            p1k = ctx.enter_context(tc.tile_pool(name="p1k", bufs=2))

            kt_r = kt_d.rearrange("(c2 o p) n -> c2 p o n", o=2, p=P)

            def load_kT(ci):
                n0 = ci * NCHUNK
                kT8 = p1k.tile([P, DT // 2, 2, NCHUNK], fp8, tag="kT8",
                               name=f"kT8_{ci}")
                for dc2 in range(DT // 2):
                    nc.sync.dma_start(
                        out=kT8[:, dc2, :, :],
                        in_=kt_r[dc2, :, :, n0:n0 + NCHUNK])
                return kT8

            # ---------------- Phase 0: key/val derivation ----------------
            # Single interleaved stream: weight transposes, then per mem
            # j-tile: transpose -> val group; every 4th tile also the ekT
            # groups and key-denominator chain for that 512-wide j-chunk.
            with tc.tile_pool(name="p0", bufs=1) as p0, \
                 tc.tile_pool(name="p0st", bufs=4) as p0st, \
                 tc.tile_pool(name="ps_vs", bufs=2, space="PSUM") as ps_vs:
                memT = [p0.tile([P, J], bf16, tag=f"memT{m}", name=f"memT{m}")
                        for m in range(MT)]
                memT8 = p0.tile([P, MT // 2, 2, J], fp8, tag="memT8")
                fkT8 = p0.tile([P, MT // 2, 2, D], fp8, tag="fkT8")
                fvT = [p0.tile([P, D], bf16, tag=f"fvT{m}", name=f"fvT{m}")
                       for m in range(MT)]
                fvb16 = p0.tile([1, D], bf16, tag="fvb16")

                fvb32 = p0st.tile([1, D], f32, tag="fvb32", bufs=1)
                nc.gpsimd.dma_start(out=fvb32,
                                    in_=fvb_d.rearrange("(a d) -> a d", a=1))
                nc.vector.tensor_copy(out=fvb16, in_=fvb32)

                # DMA order matters for the startup: first the mem j-chunk 0
                # and fv weights (gate the first val matmuls), then fk weights
                # (first needed at jt=3), then k chunk 0 inside the stream.
                kT8_pre = None
                pv0 = ps_vs.tile([1, 512], f32, tag="vs")
                pv1 = ps_vs.tile([1, 512], f32, tag="vs")
                for m in range(MT):
                    nc.sync.dma_start(out=fvT[m],
                                      in_=fvwt_d[m * P:(m + 1) * P, :])
                fkwt_r = fkwt_d.rearrange("(m2 o p) d -> m2 p o d", o=2, p=P)
                for m2 in range(MT // 2):
                    nc.sync.dma_start(out=fkT8[:, m2, :, :], in_=fkwt_r[m2])
                memt8_r = memt8_d.rearrange("(m2 o p) j -> m2 p o j",
                                            o=2, p=P)
                for m in range(MT):
                    nc.sync.dma_start(out=memT[m][:, 0:512],
                                      in_=memt16_d[m * P:(m + 1) * P, 0:512])
                for m2 in range(MT // 2):
                    nc.sync.dma_start(out=memT8[:, m2, :, 0:512],
                                      in_=memt8_r[m2, :, :, 0:512])
                for jt in range(JT):
                    if jt % 4 == 0 and jt > 0:
                        jc0 = jt // 4
                        jv = slice(jc0 * 512, (jc0 + 1) * 512)
                        for m in range(MT):
                            nc.sync.dma_start(
                                out=memT[m][:, jv],
                                in_=memt16_d[m * P:(m + 1) * P, jv])
                        for m2 in range(MT // 2):
                            nc.sync.dma_start(out=memT8[:, m2, :, jv],
                                              in_=memt8_r[m2, :, :, jv])
                    # val8[jt] = relu(sum_m memT[m].T @ fvT[m] + 1 x fv_b)
                    vt16 = p0st.tile([P, D], bf16, tag="vt16", bufs=2)
                    for dh in range(D // 512):
                        ps = ps_s.tile([P, 512], f32, tag="s")
                        for m in range(MT):
                            nc.tensor.matmul(
                                ps,
                                lhsT=memT[m][:, jt * P:(jt + 1) * P],
                                rhs=fvT[m][:, dh * 512:(dh + 1) * 512],
                                start=(m == 0), stop=False)
                        nc.tensor.matmul(
                            ps, lhsT=ones_r16,
                            rhs=fvb16[:, dh * 512:(dh + 1) * 512],
                            start=False, stop=True)
                        nc.scalar.activation(
                            out=vt16[:, dh * 512:(dh + 1) * 512], in_=ps,
                            func=AF.Relu)
                        nc.vector.tensor_scalar_max(
                            val8[:, jt // 2, jt % 2, dh * 512:(dh + 1) * 512],
                            ps, 0.0)
                    nc.tensor.matmul(pv0, lhsT=ones_c16, rhs=vt16[:, 0:512],
                                     start=(jt == 0), stop=(jt == JT - 1))
                    nc.tensor.matmul(pv1, lhsT=ones_c16, rhs=vt16[:, 512:1024],
                                     start=(jt == 0), stop=(jt == JT - 1))

                    if jt % 4 != 3:
                        continue
                    jc = jt // 4
                    # ekT8 groups for this 512-wide j-chunk (fp8 DR)
                    for dt in range(DT):
                        ps = ps_s.tile([P, 512], f32, tag="s")
                        for m2 in range(MT // 2):
                            nc.tensor.matmul(
                                ps,
                                lhsT=fkT8[:, m2, :, dt * P:(dt + 1) * P],
                                rhs=memT8[:, m2, :, jc * 512:(jc + 1) * 512],
                                start=(m2 == 0), stop=(m2 == MT // 2 - 1),
                                perf_mode=DR)
                        nc.scalar.activation(
                            out=ekT8[:, dt // 2, dt % 2,
                                     jc * 512:(jc + 1) * 512],
                            in_=ps, func=AF.Exp, bias=fkbT[:, dt:dt + 1],
                            scale=1.0)
                    # key softmax denominators -> c_col[:, jc*4:(jc+1)*4]
                    pd = ps_d.tile([1, 512], f32, tag="den")
                    for dc2 in range(DT // 2):
                        nc.tensor.matmul(
                            pd, lhsT=ones8[:, :, 0:1],
                            rhs=ekT8[:, dc2, :, jc * 512:(jc + 1) * 512],
                            start=(dc2 == 0), stop=(dc2 == DT // 2 - 1),
                            perf_mode=DR)
                    crow = p0st.tile([1, 512], f32, tag="crow", bufs=2)
                    nc.vector.tensor_copy(out=crow, in_=pd)
                    pq = ps_d.tile([P, 4], f32, tag="den")
                    for q in range(4):
                        nc.tensor.transpose(pq[:, q:q + 1],
                                            crow[:, q * P:(q + 1) * P],
                                            ident[0:1, 0:1])
                    nc.vector.reciprocal(
                        out=c_col[:, jc * 4:(jc + 1) * 4], in_=pq)
                    if jc == 0:
                        kT8_pre = load_kT(0)

                # broadcast valsum across partitions (rank-1 fp32 matmul)
                vs_row = p0.tile([1, D], f32, tag="vs_row")
                nc.vector.tensor_copy(out=vs_row[:, 0:512], in_=pv0)
                nc.vector.tensor_copy(out=vs_row[:, 512:1024], in_=pv1)
                for dh in range(D // 512):
                    pb = ps_s.tile([P, 512], f32, tag="s")
                    nc.tensor.matmul(pb, lhsT=ones_r32,
                                     rhs=vs_row[:, dh * 512:(dh + 1) * 512],
                                     start=True, stop=True)
                    nc.vector.tensor_copy(
                        out=vsum_bc[:, dh * 512:(dh + 1) * 512], in_=pb)

            # ---------------- Phase 1: attention over k rows ----------------
            with tc.tile_pool(name="p1", bufs=2) as p1, \
                 tc.tile_pool(name="p1e", bufs=2) as p1e, \
                 tc.tile_pool(name="ps_o", bufs=3, space="PSUM") as ps_o:
                for ci in range(NC_):
                    n0 = ci * NCHUNK
                    kT8 = kT8_pre if ci == 0 else load_kT(ci)

                    # scoresT (DoubleRow) -> exp -> Es = E-1 (fp8)
                    Es8 = p1e.tile([P, JT // 2, 2, NCHUNK], fp8, tag="Es8")
                    for jt in range(JT):
                        ps = ps_s.tile([P, NCHUNK], f32, tag="s")
                        for dc2 in range(DT // 2):
                            nc.tensor.matmul(
                                ps,
                                lhsT=ekT8[:, dc2, :, jt * P:(jt + 1) * P],
                                rhs=kT8[:, dc2, :, :],
                                start=(dc2 == 0), stop=(dc2 == DT // 2 - 1),
                                perf_mode=DR)
                        e16 = p1.tile([P, NCHUNK], bf16, tag="e16", bufs=3)
                        nc.scalar.activation(
                            out=e16, in_=ps, func=AF.Exp,
                            scale=c_col[:, jt:jt + 1])
                        nc.vector.tensor_scalar_add(
                            Es8[:, jt // 2, jt % 2, :], e16, -1.0)

                    # out[n-tile, dd] = (vsum + Es.T @ val) / (J + Es.T @ 1)
                    for ns in range(NS):
                        po0 = ps_o.tile([P, 512], f32, tag="o")
                        po1 = ps_o.tile([P, 512], f32, tag="o")
                        pden = ps_d.tile([P, 1], f32, tag="den")
                        for jc2 in range(JT // 2):
                            lhs = Es8[:, jc2, :, ns * P:(ns + 1) * P]
                            st_, sp_ = (jc2 == 0), (jc2 == JT // 2 - 1)
                            nc.tensor.matmul(po0, lhsT=lhs,
                                             rhs=val8[:, jc2, :, 0:512],
                                             start=st_, stop=sp_, perf_mode=DR)
                            nc.tensor.matmul(po1, lhsT=lhs,
                                             rhs=val8[:, jc2, :, 512:1024],
                                             start=st_, stop=sp_, perf_mode=DR)
                            nc.tensor.matmul(pden, lhsT=lhs,
                                             rhs=ones8[:, :, 0:1],
                                             start=st_, stop=sp_, perf_mode=DR)
                        rv = p1.tile([P, 1], f32, tag="rv")
                        nc.vector.tensor_scalar_add(rv, pden, float(J))
                        nc.vector.reciprocal(out=rv, in_=rv)
                        for dh, po in ((0, po0), (1, po1)):
                            osb = p1.tile([P, 512], f32, tag="osb")
                            nc.vector.tensor_add(
                                osb, po, vsum_bc[:, dh * 512:(dh + 1) * 512])
                            nc.vector.tensor_scalar_mul(osb, osb, rv)
                            nc.sync.dma_start(
                                out=out_d[n0 + ns * P:n0 + (ns + 1) * P,
                                          dh * 512:(dh + 1) * 512],
                                in_=osb)

    nc.compile()
    return nc


def _get_nc():
    if "nc" not in _CACHE:
        _CACHE["nc"] = _build()
    return _CACHE["nc"]


def kernel(**inputs) -> np.ndarray:
    from concourse.bass_utils import run_bass_kernel_spmd

    k = np.asarray(inputs["k"], dtype=np.float32)
    mem = np.asarray(inputs["mem"], dtype=np.float32)
    fk_w = np.asarray(inputs["fk_w"], dtype=np.float32)
    fk_b = np.ascontiguousarray(np.asarray(inputs["fk_b"], dtype=np.float32))
    fv_w = np.asarray(inputs["fv_w"], dtype=np.float32)
    fv_b = np.ascontiguousarray(np.asarray(inputs["fv_b"], dtype=np.float32))
    ident = np.eye(P, dtype=np.float32)

    # host-side layout prep: pre-transpose (contraction dims on SBUF
    # partitions) and pre-cast to the on-chip compute dtypes so DMA can
    # write straight into the persistent SBUF tiles
    import ml_dtypes
    bf16 = ml_dtypes.bfloat16
    f8 = ml_dtypes.float8_e4m3
    memt = np.ascontiguousarray(mem.T)
    memt16 = memt.astype(bf16)
    memt8 = memt16.astype(np.float32).astype(f8)
    fkwt8 = np.ascontiguousarray(fk_w.T).astype(bf16).astype(np.float32).astype(f8)
    fvwt16 = np.ascontiguousarray(fv_w.T).astype(bf16)

    nc = _get_nc()
    in_maps = []
    for c in range(NCORES):
        in_maps.append({
            "kt8": np.ascontiguousarray(k[c * S:(c + 1) * S].T).astype(f8),
            "memt16": memt16, "memt8": memt8, "fkwt8": fkwt8, "fk_b": fk_b,
            "fvwt16": fvwt16, "fv_b": fv_b, "ident": ident,
        })
    res = run_bass_kernel_spmd(nc, in_maps, core_ids=list(range(NCORES)),
                               **_CACHE.get("run_kwargs", {}))
    _CACHE["last_result"] = res
    return np.concatenate([res.results[c]["out"] for c in range(NCORES)],
                          axis=0)

